# revision 10
# baseline (speedup 1.0000x reference)
import sys

import numpy as np

if "/opt/trn_rl_repo" not in sys.path:
    sys.path.insert(0, "/opt/trn_rl_repo")

import concourse.bacc as bacc
import concourse.bass as bass
import concourse.mybir as mybir
import concourse.tile as tile
from concourse.bass_utils import run_bass_kernel_spmd

# Problem constants (hardcoded per harness contract)
B, C, K = 32768, 1000, 5
N_CORES = 8
ROWS = B // N_CORES          # 4096 rows per core
P = 128                      # partitions
NT = ROWS // P               # 32 row-tiles per core
TB = 4                       # tiles per wave (per indirect_copy batch)
NW = NT // TB                # 8 waves
GCOL = 16 * K                # 80 gather output cols per row-tile
FP32 = mybir.dt.float32


def _build_kernel():
    nc = bacc.Bacc()
    x = nc.declare_dram_parameter("x", [ROWS, C], FP32, isOutput=False)
    idx = nc.declare_dram_parameter("idx", [P, NT * K], mybir.dt.uint16, isOutput=False)
    msk = nc.declare_dram_parameter("msk", [P, GCOL], FP32, isOutput=False)
    out = nc.declare_dram_parameter("out", [1, 1], FP32, isOutput=True)

    with tile.TileContext(nc) as tc:
        with (
            tc.tile_pool(name="wave", bufs=3) as wave_pool,
            tc.tile_pool(name="persist", bufs=1) as pp,
        ):
            g_all = pp.tile([P, NT * GCOL], FP32)      # gathered raw logits
            idx_sb = pp.tile([P, NT * K], mybir.dt.uint16)
            msk_sb = pp.tile([P, GCOL], FP32)
            wm = pp.tile([P, NT * GCOL], FP32)         # masked exp(gathered)
            denom = pp.tile([P, NT], FP32)
            numer = pp.tile([P, NT], FP32)
            rec = pp.tile([P, NT], FP32)
            loss = pp.tile([P, NT], FP32)
            total = pp.tile([1, 1], FP32)

            nc.sync.dma_start(out=idx_sb[:], in_=idx[:])
            nc.sync.dma_start(out=msk_sb[:], in_=msk[:])

            # Streaming waves: DMA -> gather -> exp(+denominator accumulate)
            for wv_i in range(NW):
                wtile = wave_pool.tile([P, TB * C], FP32)
                xw = x[wv_i * TB * P:(wv_i + 1) * TB * P, :].rearrange(
                    "(t p) c -> p t c", p=P
                )
                nc.sync.dma_start(
                    out=wtile[:].rearrange("p (t c) -> p t c", t=TB), in_=xw
                )
                nc.gpsimd.indirect_copy(
                    out=g_all[:, wv_i * TB * GCOL:(wv_i + 1) * TB * GCOL],
                    data=wtile[:],
                    idxs=idx_sb[:, wv_i * TB * K:(wv_i + 1) * TB * K],
                    i_know_ap_gather_is_preferred=True,
                )
                for tt in range(TB):
                    t = wv_i * TB + tt
                    nc.scalar.activation(
                        out=wtile[:, tt * C:(tt + 1) * C],
                        in_=wtile[:, tt * C:(tt + 1) * C],
                        func=mybir.ActivationFunctionType.Exp,
                        accum_out=denom[:, t:t + 1],
                    )

            # Numerators: exp the gathered logits, select each row's own
            # entries (position mask) with dedup weights, reduce per tile.
            nc.scalar.activation(
                out=g_all[:], in_=g_all[:], func=mybir.ActivationFunctionType.Exp,
            )
            # wm[p, t, g] = exp(g_all)[p, t, g] * msk[p, g] (msk broadcast over t;
            # the mask keeps only each partition's own gathered entries)
            m3 = msk_sb[:].rearrange("p (k q) -> p k q", k=K)
            m4 = bass.AP(m3.tensor, m3.offset, [m3.ap[0], [0, NT], m3.ap[1], m3.ap[2]])
            wm4 = wm[:].rearrange("p (t k q) -> p t k q", k=K, q=16)
            g4 = g_all[:].rearrange("p (t k q) -> p t k q", k=K, q=16)
            nc.vector.tensor_tensor(out=wm4, in0=g4, in1=m4, op=mybir.AluOpType.mult)
            nc.vector.tensor_reduce(
                out=numer[:],
                in_=wm[:].rearrange("p (t g) -> p t g", g=GCOL),
                axis=mybir.AxisListType.X,
                op=mybir.AluOpType.add,
            )

            nc.vector.reciprocal(out=rec[:], in_=denom[:])
            nc.vector.tensor_tensor(
                out=loss[:], in0=numer[:], in1=rec[:], op=mybir.AluOpType.mult,
            )
            lsum = pp.tile([P, 1], FP32)
            red = pp.tile([P, 1], FP32)
            nc.vector.tensor_reduce(
                out=lsum[:], in_=loss[:],
                axis=mybir.AxisListType.X, op=mybir.AluOpType.add,
            )
            import concourse.bass_isa as bass_isa
            nc.gpsimd.partition_all_reduce(
                out_ap=red[:], in_ap=lsum[:], channels=P,
                reduce_op=bass_isa.ReduceOp.add,
            )
            nc.vector.tensor_copy(out=total[:], in_=red[:1, :])
            nc.sync.dma_start(out=out[:], in_=total[:])

    if not nc.is_finalized():
        nc.finalize()
    return nc


_CACHE = {}


def _prep_inputs(outputs, complementary_labels):
    outputs = np.ascontiguousarray(outputs, dtype=np.float32)
    labels = np.asarray(complementary_labels).astype(np.int64)

    # Position mask: out col i (within a row-tile's 80) holds data for the
    # partition whose p%16 == i%16; k = i//16.
    msk = (np.arange(P)[:, None] % 16 == np.arange(GCOL)[None, :] % 16)
    msk = np.ascontiguousarray(msk, dtype=np.float32)

    in_maps = []
    for c in range(N_CORES):
        x_c = outputs[c * ROWS:(c + 1) * ROWS]
        lab = labels[c * ROWS:(c + 1) * ROWS].reshape(NT, P, K)
        off = (np.arange(NT) % TB * C)[:, None, None]
        idxv = (lab + off).astype(np.uint16)               # [NT, P, K]
        # idx[p, w*TB*K + tt*K + k] for wave w, tile-in-wave tt
        idx_c = np.ascontiguousarray(
            idxv.reshape(NW, TB, P, K).transpose(2, 0, 1, 3).reshape(P, NT * K)
        )
        in_maps.append({"x": np.ascontiguousarray(x_c), "idx": idx_c, "msk": msk})
    return in_maps


def kernel(outputs, complementary_labels):
    if "nc" not in _CACHE:
        _CACHE["nc"] = _build_kernel()
    nc = _CACHE["nc"]
    in_maps = _prep_inputs(outputs, complementary_labels)
    res = run_bass_kernel_spmd(nc, in_maps, list(range(N_CORES)))
    total = 0.0
    for r in res.results:
        total += float(np.asarray(r["out"]).reshape(-1)[0])
    return np.array(total / B, dtype=np.float32)


# revision 11
# speedup vs baseline: 12.0935x; 12.0935x over previous
import sys

import numpy as np

if "/opt/trn_rl_repo" not in sys.path:
    sys.path.insert(0, "/opt/trn_rl_repo")

import concourse.bacc as bacc
import concourse.bass as bass
import concourse.mybir as mybir
import concourse.tile as tile
from concourse.bass_utils import run_bass_kernel_spmd

# Problem constants (hardcoded per harness contract)
B, C, K = 32768, 1000, 5
N_CORES = 8
ROWS = B // N_CORES          # 4096 rows per core
P = 128                      # partitions
NT = ROWS // P               # 32 row-tiles per core
TB = 4                       # tiles per wave (per indirect_copy batch)
NW = NT // TB                # 8 waves
GCOL = 16 * K                # 80 gather output cols per row-tile
FP32 = mybir.dt.float32


def _build_kernel(loop_n=None):
    nc = bacc.Bacc()
    x = nc.declare_dram_parameter("x", [ROWS, C], FP32, isOutput=False)
    idx = nc.declare_dram_parameter("idx", [P, NT * K], mybir.dt.uint16, isOutput=False)
    msk = nc.declare_dram_parameter("msk", [P, GCOL], FP32, isOutput=False)
    out = nc.declare_dram_parameter("out", [1, 1], FP32, isOutput=True)

    with tile.TileContext(nc) as tc:
        from contextlib import ExitStack
        with ExitStack() as stack:
            wave_pool = stack.enter_context(tc.tile_pool(name="wave", bufs=3))
            pp = stack.enter_context(tc.tile_pool(name="persist", bufs=1))
            if loop_n is not None:
                stack.enter_context(tc.For_i(0, loop_n, 1))
            g_all = pp.tile([P, NT * GCOL], FP32)      # gathered raw logits
            idx_sb = pp.tile([P, NT * K], mybir.dt.uint16)
            msk_sb = pp.tile([P, GCOL], FP32)
            wm = pp.tile([P, NT * GCOL], FP32)         # masked exp(gathered)
            denom = pp.tile([P, NT], FP32)
            numer = pp.tile([P, NT], FP32)
            rec = pp.tile([P, NT], FP32)
            loss = pp.tile([P, NT], FP32)
            total = pp.tile([1, 1], FP32)

            nc.sync.dma_start(out=idx_sb[:], in_=idx[:])
            nc.sync.dma_start(out=msk_sb[:], in_=msk[:])

            # Streaming waves: DMA -> gather -> exp(+denominator accumulate)
            for wv_i in range(NW):
                wtile = wave_pool.tile([P, TB * C], FP32)
                xw = x[wv_i * TB * P:(wv_i + 1) * TB * P, :].rearrange(
                    "(t p) c -> p t c", p=P
                )
                nc.sync.dma_start(
                    out=wtile[:].rearrange("p (t c) -> p t c", t=TB), in_=xw
                )
                nc.gpsimd.indirect_copy(
                    out=g_all[:, wv_i * TB * GCOL:(wv_i + 1) * TB * GCOL],
                    data=wtile[:],
                    idxs=idx_sb[:, wv_i * TB * K:(wv_i + 1) * TB * K],
                    i_know_ap_gather_is_preferred=True,
                )
                for tt in range(TB):
                    t = wv_i * TB + tt
                    nc.scalar.activation(
                        out=wtile[:, tt * C:(tt + 1) * C],
                        in_=wtile[:, tt * C:(tt + 1) * C],
                        func=mybir.ActivationFunctionType.Exp,
                        accum_out=denom[:, t:t + 1],
                    )

            # Numerators: exp the gathered logits, select each row's own
            # entries (position mask) with dedup weights, reduce per tile.
            nc.scalar.activation(
                out=g_all[:], in_=g_all[:], func=mybir.ActivationFunctionType.Exp,
            )
            # wm[p, t, g] = exp(g_all)[p, t, g] * msk[p, g] (msk broadcast over t;
            # the mask keeps only each partition's own gathered entries)
            m3 = msk_sb[:].rearrange("p (k q) -> p k q", k=K)
            m4 = bass.AP(m3.tensor, m3.offset, [m3.ap[0], [0, NT], m3.ap[1], m3.ap[2]])
            wm4 = wm[:].rearrange("p (t k q) -> p t k q", k=K, q=16)
            g4 = g_all[:].rearrange("p (t k q) -> p t k q", k=K, q=16)
            nc.vector.tensor_tensor(out=wm4, in0=g4, in1=m4, op=mybir.AluOpType.mult)
            nc.vector.tensor_reduce(
                out=numer[:],
                in_=wm[:].rearrange("p (t g) -> p t g", g=GCOL),
                axis=mybir.AxisListType.X,
                op=mybir.AluOpType.add,
            )

            nc.vector.reciprocal(out=rec[:], in_=denom[:])
            nc.vector.tensor_tensor(
                out=loss[:], in0=numer[:], in1=rec[:], op=mybir.AluOpType.mult,
            )
            lsum = pp.tile([P, 1], FP32)
            red = pp.tile([P, 1], FP32)
            nc.vector.tensor_reduce(
                out=lsum[:], in_=loss[:],
                axis=mybir.AxisListType.X, op=mybir.AluOpType.add,
            )
            import concourse.bass_isa as bass_isa
            nc.gpsimd.partition_all_reduce(
                out_ap=red[:], in_ap=lsum[:], channels=P,
                reduce_op=bass_isa.ReduceOp.add,
            )
            nc.vector.tensor_copy(out=total[:], in_=red[:1, :])
            nc.sync.dma_start(out=out[:], in_=total[:])

    if not nc.is_finalized():
        nc.finalize()
    return nc


_CACHE = {}


def _prep_inputs(outputs, complementary_labels):
    outputs = np.ascontiguousarray(outputs, dtype=np.float32)
    labels = np.asarray(complementary_labels).astype(np.int64)

    # Position mask: out col i (within a row-tile's 80) holds data for the
    # partition whose p%16 == i%16; k = i//16.
    msk = (np.arange(P)[:, None] % 16 == np.arange(GCOL)[None, :] % 16)
    msk = np.ascontiguousarray(msk, dtype=np.float32)

    in_maps = []
    for c in range(N_CORES):
        x_c = outputs[c * ROWS:(c + 1) * ROWS]
        lab = labels[c * ROWS:(c + 1) * ROWS].reshape(NT, P, K)
        off = (np.arange(NT) % TB * C)[:, None, None]
        idxv = (lab + off).astype(np.uint16)               # [NT, P, K]
        # idx[p, w*TB*K + tt*K + k] for wave w, tile-in-wave tt
        idx_c = np.ascontiguousarray(
            idxv.reshape(NW, TB, P, K).transpose(2, 0, 1, 3).reshape(P, NT * K)
        )
        in_maps.append({"x": np.ascontiguousarray(x_c), "idx": idx_c, "msk": msk})
    return in_maps


def kernel(outputs, complementary_labels):
    if "nc" not in _CACHE:
        _CACHE["nc"] = _build_kernel()
    nc = _CACHE["nc"]
    in_maps = _prep_inputs(outputs, complementary_labels)
    res = run_bass_kernel_spmd(nc, in_maps, list(range(N_CORES)))
    total = 0.0
    for r in res.results:
        total += float(np.asarray(r["out"]).reshape(-1)[0])
    return np.array(total / B, dtype=np.float32)
